# revision 2
# baseline (speedup 1.0000x reference)
"""AGDNConv (GAT transition + K-hop diffusion + hop attention) on 8 TRN2 NeuronCores.

Strategy (dst-sharded graph parallel):
- Nodes sharded contiguously: core c owns nodes [c*6250, (c+1)*6250).
- Stage 1 (per core): fc matmul (H0^T = W @ feat^T), attention projections
  el/er, H0 rows assembled as bf16 table rows [H0 | el | pad] (768B).
- Per-hop tables (H_k rows for ALL nodes, bf16) are replicated via AllGather,
  split in two halves (A/B by local index) so the next hop's gathers can
  overlap the second half's collective.
- Aggregation per hop: per 128-dst tile, dma_gather the src rows of all its
  (dst-sorted, subwindow-aligned, 128-padded) edge slots, weight rows by
  w = exp(leaky(el[src]+er[dst])) on DVE, and segment-sum via PE matmuls
  with per-chunk 0/1 selection matrices (fp8) into PSUM dst slices.
  Softmax denominators come for free as an extra w column in the hop-1
  matmul; normalization is a per-row multiply by 1/s afterwards.
- Hop outputs (+pos_emb) accumulate into an hstack buffer; the final stage
  computes hop attention (softmax over K+1) and the weighted sum on DVE.

All index manipulation (sharding, sorting, slot layout, padding) happens on
the host; all floating-point math runs on device.
"""

import os
import numpy as np
import ml_dtypes

import concourse.bacc as bacc
import concourse.bass as bass
import concourse.mybir as mybir
import concourse.tile as tile
from concourse.bass_utils import run_bass_kernel_spmd
from concourse.masks import make_identity

# ---------------- problem geometry (hardcoded per spec) ----------------
N, E, IN, H, D, K = 50000, 800000, 256, 4, 64, 3
HD = H * D                     # 256
NC = 8
NLOC = N // NC                 # 6250
HALF_DATA = NLOC // 2          # 3125
HALF = 3200                    # padded rows per half (25 tiles of 128)
NLOCP = 2 * HALF               # 6400
NT = NLOCP // 128              # 50 n-tiles
NT_A = HALF // 128             # 25 (tiles 0..24 are half A)
TROWS = NC * HALF              # 25600 rows per half-table (int16-safe)
ELEM1 = 384                    # hop-1 table row elems (256 H0 + 4 el + pad)
ELEM = 256
ER_ELEM = 128                  # er table row elems (256B rows)
ZROW = HALF_DATA               # zero row (per-core local pad row) in both halves
NEG_SLOPE = 0.2
MASK_NEG = -1.0e30

P = 128
SUBW = 64
f32 = mybir.dt.float32
bf16 = mybir.dt.bfloat16
fp8 = mybir.dt.float8e4
i16 = mybir.dt.int16


# ---------------- host-side preparation ----------------

def _prep(feat, src, dst, W_src, attn_l, attn_r, hop_attn_l, hop_attn_r,
          pos_emb, bias):
    src = np.asarray(src).astype(np.int64)
    dst = np.asarray(dst).astype(np.int64)
    feat = np.asarray(feat, dtype=np.float32)

    core = dst // NLOC
    j = dst % NLOC
    jp = j + 75 * (j >= HALF_DATA)            # padded local coordinate of dst
    t_of = jp >> 7
    q_of = (jp >> 6) & 1
    dl = jp & 63

    sc = src // NLOC
    sj = src % NLOC
    half = (sj >= HALF_DATA).astype(np.int64)
    tblidx = sc * HALF + sj - HALF_DATA * half   # index within half-table
    er_idx = jp                                   # dst local padded coord

    # group = (t, half, q) per edge; chunk order within a tile is
    # (q0,lo),(q1,lo),(q0,hi),(q1,hi) => group sort key half*2 + q
    NG = NT * 4
    g_of = t_of * 4 + half * 2 + q_of

    # per-core group counts -> global structure (max over cores)
    counts = np.zeros((NC, NG), np.int64)
    for c in range(NC):
        m = core == c
        counts[c] = np.bincount(g_of[m], minlength=NG)
    nch_g = (np.max(counts, axis=0) + 127) // 128          # chunks per group
    cap_g = nch_g * 128                                     # slots per group
    gbase = np.zeros(NG + 1, np.int64)
    np.cumsum(cap_g, out=gbase[1:])
    NSLOT = int(gbase[-1])
    NCHUNK = NSLOT // 128
    assert NSLOT % 128 == 0

    # tile-level structure for the device build
    # for each tile: list of (qoff, half) per chunk, slot range, lo-chunk count
    tiles = []
    for t in range(NT):
        chunks = []
        for hq in range(4):                  # (half,q) = (0,0),(0,1),(1,0),(1,1)
            hf, q = hq >> 1, hq & 1
            g = t * 4 + hf * 2 + q
            for _ in range(int(nch_g[g])):
                chunks.append((q * SUBW, hf))
        s0 = int(gbase[t * 4])
        nlo = int(nch_g[t * 4] + nch_g[t * 4 + 1])
        tiles.append({
            "s0": s0,
            "nch": len(chunks),
            "nlo": nlo,
            "chunks": chunks,
            "c0": s0 // 128,
        })

    # per-core slot arrays
    idx_main = np.full((NC, NSLOT), ZROW, np.int16)
    idx_er = np.full((NC, NSLOT), ZROW, np.int16)
    dl_flat = np.zeros((NC, NSLOT), np.float32)
    mask_flat = np.full((NC, NSLOT), MASK_NEG, np.float32)

    order_key = g_of
    for c in range(NC):
        m = np.nonzero(core == c)[0]
        o = m[np.argsort(order_key[m], kind="stable")]
        g_sorted = g_of[o]
        gc = counts[c]
        # rank within group
        gstart = np.zeros(NG + 1, np.int64)
        np.cumsum(gc, out=gstart[1:])
        rank = np.arange(len(o)) - gstart[g_sorted]
        slot = gbase[g_sorted] + rank
        idx_main[c, slot] = tblidx[o]
        idx_er[c, slot] = er_idx[o]
        dl_flat[c, slot] = dl[o]
        mask_flat[c, slot] = 0.0

    # wrap idx arrays [(s%16), s//16] replicated to 128 partitions
    def wrap16(a):
        w = a.reshape(NSLOT // 16, 16).T            # [16, NSLOT//16]
        return np.tile(w, (8, 1)).copy()            # [128, NSLOT//16]

    # dl/mask: [128, NCHUNK] with slot (s%128, s//128)
    def wrap128(a):
        return a.reshape(NCHUNK, 128).T.copy()

    # feat^T padded per core
    featT = np.zeros((NC, IN, NLOCP), np.float32)
    for c in range(NC):
        fc = feat[c * NLOC:(c + 1) * NLOC]          # [6250, 256]
        featT[c, :, :HALF_DATA] = fc[:HALF_DATA].T
        featT[c, :, HALF:HALF + HALF_DATA] = fc[HALF_DATA:].T

    WT = np.ascontiguousarray(np.asarray(W_src, np.float32).T)    # [IN, HD]
    al = np.asarray(attn_l, np.float32).reshape(H, D)
    ar = np.asarray(attn_r, np.float32).reshape(H, D)
    AlAr = np.zeros((HD, 2 * H), np.float32)
    for h in range(H):
        AlAr[h * D:(h + 1) * D, h] = al[h]
        AlAr[h * D:(h + 1) * D, H + h] = ar[h]

    hopl = np.asarray(hop_attn_l, np.float32).reshape(H * D)
    hopr = np.asarray(hop_attn_r, np.float32).reshape(H * D)
    hopl_r = np.broadcast_to(hopl, (P, HD)).copy()
    hopr_r = np.broadcast_to(hopr, (P, HD)).copy()
    pos = np.asarray(pos_emb, np.float32).reshape(H, K + 1, D)
    pos_flat = np.transpose(pos, (1, 0, 2)).reshape(K + 1, HD)    # [k, h*64+d]
    pos_r = np.broadcast_to(pos_flat, (P, K + 1, HD)).copy()
    bias_r = np.broadcast_to(np.asarray(bias, np.float32).reshape(HD), (P, HD)).copy()
    iota = np.broadcast_to(np.arange(SUBW, dtype=np.float32), (P, SUBW)).copy()

    in_maps = []
    for c in range(NC):
        in_maps.append({
            "featT": featT[c],
            "WT": WT,
            "AlAr": AlAr,
            "hopl": hopl_r,
            "hopr": hopr_r,
            "pos": pos_r,
            "bias": bias_r,
            "iota": iota,
            "idxm": wrap16(idx_main[c]),
            "idxe": wrap16(idx_er[c]),
            "dlt": wrap128(dl_flat[c]),
            "maskt": wrap128(mask_flat[c]),
        })
    return in_maps, tiles, NSLOT, NCHUNK


# ---------------- device kernel ----------------

def _build(tiles, NSLOT, NCHUNK):
    nc = bacc.Bacc("TRN2", debug=False)

    featT_in = nc.dram_tensor("featT", [IN, NLOCP], f32, kind="ExternalInput")
    WT_in = nc.dram_tensor("WT", [IN, HD], f32, kind="ExternalInput")
    AlAr_in = nc.dram_tensor("AlAr", [HD, 2 * H], f32, kind="ExternalInput")
    hopl_in = nc.dram_tensor("hopl", [P, HD], f32, kind="ExternalInput")
    hopr_in = nc.dram_tensor("hopr", [P, HD], f32, kind="ExternalInput")
    pos_in = nc.dram_tensor("pos", [P, K + 1, HD], f32, kind="ExternalInput")
    bias_in = nc.dram_tensor("bias", [P, HD], f32, kind="ExternalInput")
    iota_in = nc.dram_tensor("iota", [P, SUBW], f32, kind="ExternalInput")
    idxm_in = nc.dram_tensor("idxm", [P, NSLOT // 16], i16, kind="ExternalInput")
    idxe_in = nc.dram_tensor("idxe", [P, NSLOT // 16], i16, kind="ExternalInput")
    dlt_in = nc.dram_tensor("dlt", [P, NCHUNK], f32, kind="ExternalInput")
    maskt_in = nc.dram_tensor("maskt", [P, NCHUNK], f32, kind="ExternalInput")
    out_ext = nc.dram_tensor("out", [NLOCP, HD], f32, kind="ExternalOutput")
    debug = bool(int(os.environ.get("AGDN_DEBUG", "0")))
    if debug:
        dbg_hs = nc.dram_tensor("dbg_hs", [NLOCP, K + 1, HD], mybir.dt.bfloat16,
                                kind="ExternalOutput")
        dbg_w = nc.dram_tensor("dbg_w", [P, 1024, H], mybir.dt.bfloat16,
                               kind="ExternalOutput")
        dbg_recip = nc.dram_tensor("dbg_recip", [P, NT, H], f32, kind="ExternalOutput")

    rg = [list(range(NC))]
    maxch_half = max(max(tl["nlo"], tl["nch"] - tl["nlo"]) for tl in tiles)
    maxch = max(tl["nch"] for tl in tiles)

    with tile.TileContext(nc) as tc:
        with (
            tc.tile_pool(name="dram", bufs=1, space="DRAM") as dram,
            tc.tile_pool(name="pers", bufs=1) as pers,
            tc.tile_pool(name="work", bufs=2) as work,
            tc.tile_pool(name="gat", bufs=2) as gat,
            tc.tile_pool(name="psum", bufs=2, space="PSUM") as psum,
            tc.tile_pool(name="apsum", bufs=3, space="PSUM") as apsum,
        ):
            # ---- persistent DRAM ----
            shardA = [dram.tile([HALF, ELEM1 if k == 1 else ELEM], bf16,
                                tag=f"shA{k}", name=f"shA{k}") for k in (1, 2, 3)]
            shardB = [dram.tile([HALF, ELEM1 if k == 1 else ELEM], bf16,
                                tag=f"shB{k}", name=f"shB{k}") for k in (1, 2, 3)]
            tblA = [dram.tile([TROWS, ELEM1 if k == 1 else ELEM], bf16, addr_space="Shared",
                              tag=f"tbA{k}", name=f"tbA{k}") for k in (1, 2, 3)]
            tblB = [dram.tile([TROWS, ELEM1 if k == 1 else ELEM], bf16, addr_space="Shared",
                              tag=f"tbB{k}", name=f"tbB{k}") for k in (1, 2, 3)]
            er_tbl = dram.tile([NLOCP, ER_ELEM], bf16, tag="ertbl")
            hstack = dram.tile([NLOCP, K + 1, HD], bf16, tag="hstack")

            # ---- persistent SBUF ----
            sel_all = pers.tile([P, NCHUNK, SUBW], fp8, tag="sel")
            w_all = pers.tile([P, NCHUNK, H], bf16, tag="w")
            recip_all = pers.tile([P, NT, H], f32, tag="recip")
            idxm = pers.tile([P, NSLOT // 16], i16, tag="idxm")
            hopl_s = pers.tile([P, HD], f32, tag="hopl")
            hopr_s = pers.tile([P, HD], f32, tag="hopr")
            pos_s = pers.tile([P, K + 1, HD], f32, tag="pos")
            bias_s = pers.tile([P, HD], f32, tag="bias")
            iota_s = pers.tile([P, SUBW], f32, tag="iota")
            dlt_s = pers.tile([P, NCHUNK], f32, tag="dlt")
            maskt_s = pers.tile([P, NCHUNK], f32, tag="maskt")
            ident = pers.tile([P, P], bf16, tag="ident")

            nc.sync.dma_start(out=idxm[:], in_=idxm_in[:])
            nc.sync.dma_start(out=hopl_s[:], in_=hopl_in[:])
            nc.sync.dma_start(out=hopr_s[:], in_=hopr_in[:])
            nc.sync.dma_start(out=pos_s[:], in_=pos_in[:])
            nc.sync.dma_start(out=bias_s[:], in_=bias_in[:])
            nc.sync.dma_start(out=iota_s[:], in_=iota_in[:])
            nc.sync.dma_start(out=dlt_s[:], in_=dlt_in[:])
            nc.sync.dma_start(out=maskt_s[:], in_=maskt_in[:])
            make_identity(nc, ident[:])

            # ---- sel build (batched is_equal) ----
            BQ = 32
            for c0 in range(0, NCHUNK, BQ):
                B = min(BQ, NCHUNK - c0)
                nc.vector.tensor_tensor(
                    out=sel_all[:, c0:c0 + B, :],
                    in0=dlt_s[:, c0:c0 + B].to_broadcast([P, B, SUBW]),
                    in1=iota_s[:].unsqueeze(1).to_broadcast([P, B, SUBW]),
                    op=mybir.AluOpType.is_equal,
                )

            # ---- stage 1: fc + el/er + table1/er-table/hstack0 ----
            WT_s = pers.tile([P, 2, HD], f32, tag="wts")     # [k-chunk, o]
            AlAr_s = pers.tile([P, 2, 2 * H], f32, tag="alar")
            nc.sync.dma_start(out=WT_s[:], in_=WT_in[:].rearrange("(a p) o -> p a o", p=P))
            nc.sync.dma_start(out=AlAr_s[:], in_=AlAr_in[:].rearrange("(a p) o -> p a o", p=P))
            AlAr_bf = pers.tile([P, 2, 2 * H], bf16, tag="alarbf")
            nc.vector.tensor_copy(out=AlAr_bf[:], in_=AlAr_s[:])

            NB = 512

            def stage1_block(nb):
                n0 = nb * NB
                w_ = min(NB, NLOCP - n0)
                ft = [work.tile([P, NB], f32, tag=f"ft{i}", name=f"ft{i}") for i in range(2)]
                for kc in range(2):
                    nc.sync.dma_start(out=ft[kc][:, :w_],
                                      in_=featT_in[kc * P:(kc + 1) * P, n0:n0 + w_])
                h0t_sb = work.tile([P, 2, NB], bf16, tag="h0t")
                for oh in range(2):
                    h0t_ps = psum.tile([P, NB], f32, space="PSUM", tag="h0tp")
                    for kc in range(2):
                        nc.tensor.matmul(
                            h0t_ps[:, :w_],
                            lhsT=WT_s[:, kc, oh * P:(oh + 1) * P],
                            rhs=ft[kc][:, :w_],
                            start=(kc == 0), stop=(kc == 1),
                        )
                    nc.vector.tensor_copy(out=h0t_sb[:, oh, :w_], in_=h0t_ps[:, :w_])
                for sub in range(w_ // P):
                    t = (n0 + sub * P) // P
                    eler_ps = psum.tile([P, 2 * H], f32, space="PSUM", tag="elerp", bufs=1)
                    for oh in range(2):
                        nc.tensor.matmul(
                            eler_ps[:],
                            lhsT=h0t_sb[:, oh, sub * P:(sub + 1) * P],
                            rhs=AlAr_bf[:, oh, :],
                            start=(oh == 0), stop=(oh == 1),
                        )
                    h0row_ps = psum.tile([P, HD], bf16, space="PSUM", tag="h0rp")
                    for oh in range(2):
                        nc.tensor.transpose(
                            out=h0row_ps[:, oh * P:(oh + 1) * P],
                            in_=h0t_sb[:, oh, sub * P:(sub + 1) * P],
                            identity=ident[:],
                        )
                    row_sb = work.tile([P, ELEM1], bf16, tag="row1")
                    nc.vector.tensor_copy(out=row_sb[:, 0:HD], in_=h0row_ps[:])
                    nc.vector.tensor_copy(out=row_sb[:, HD:HD + H], in_=eler_ps[:, 0:H])
                    nc.vector.memset(row_sb[:, HD + H:], 0.0)
                    sh, r0 = (shardA[0], t * P) if t < NT_A else (shardB[0], t * P - HALF)
                    nc.sync.dma_start(out=sh[r0:r0 + P, :], in_=row_sb[:])
                    # hstack k=0: H0 + pos[0]
                    hs0 = work.tile([P, HD], bf16, tag="hs")
                    nc.vector.tensor_tensor(out=hs0[:], in0=h0row_ps[:],
                                            in1=pos_s[:, 0, :], op=mybir.AluOpType.add)
                    nc.sync.dma_start(out=hstack[t * P:(t + 1) * P, 0, :], in_=hs0[:])
                    if debug:
                        nc.sync.dma_start(out=dbg_hs[t * P:(t + 1) * P, 0, :], in_=hs0[:])
                    # er table rows
                    er_bf = work.tile([P, H], bf16, tag="erbf")
                    nc.vector.tensor_copy(out=er_bf[:], in_=eler_ps[:, H:2 * H])
                    nc.sync.dma_start(out=er_tbl[t * P:(t + 1) * P, 0:H], in_=er_bf[:])

            for nb in range(NLOCP // NB + (1 if NLOCP % NB else 0)):
                stage1_block(nb)
                if nb * NB < HALF <= (nb + 1) * NB:
                    nc.gpsimd.collective_compute(
                        "AllGather", mybir.AluOpType.bypass, replica_groups=rg,
                        ins=[shardA[0].opt()], outs=[tblA[0].opt()])
            nc.gpsimd.collective_compute(
                "AllGather", mybir.AluOpType.bypass, replica_groups=rg,
                ins=[shardB[0].opt()], outs=[tblB[0].opt()])

            # ---- hops ----
            def hop(k):
                """k = 1,2,3: read tbl[k-1], write shard[k]/tbl[k] (k<3), hstack k."""
                el1 = ELEM1 if k == 1 else ELEM
                tA, tB = tblA[k - 1], tblB[k - 1]
                ncol = HD + H if k == 1 else HD
                for t in range(NT):
                    tl = tiles[t]
                    nchk, nlo = tl["nch"], tl["nlo"]
                    nhi = nchk - nlo
                    c0, s0 = tl["c0"], tl["s0"]
                    ps = apsum.tile([P, 320], f32, space="PSUM", tag="agg")
                    nc.vector.memset(ps[:, :ncol], 0.0)
                    if nchk:
                        g_lo = gat.tile([P, maxch_half, el1], bf16, tag="glo")
                        g_hi = gat.tile([P, maxch_half, el1], bf16, tag="ghi")
                        GCAP = 8
                        def gsplit(dst, tbl, base_slot, nch_piece, elem):
                            for b0 in range(0, nch_piece, GCAP):
                                nn = min(GCAP, nch_piece - b0)
                                a0 = base_slot + b0 * P
                                nc.gpsimd.dma_gather(
                                    dst[:, b0:b0 + nn, :], tbl[:],
                                    idxm[:, a0 // 16:(a0 + nn * P) // 16],
                                    nn * P, nn * P, elem)
                        if nlo and not no_gat:
                            gsplit(g_lo, tA, s0, nlo, el1)
                        if nhi and not no_gat:
                            gsplit(g_hi, tB, s0 + nlo * P, nhi, el1)
                        if k == 1 and not no_w:
                            # gather er rows + build w
                            idxe_t = work.tile([P, maxch * 8], i16, tag="idxe")
                            nc.sync.dma_start(
                                out=idxe_t[:, :nchk * 8],
                                in_=idxe_in[:, s0 // 16:(s0 + nchk * P) // 16])
                            er_g = gat.tile([P, maxch, ER_ELEM], bf16, tag="erg")
                            for b0 in range(0, nchk, 8):
                                nn = min(8, nchk - b0)
                                nc.gpsimd.dma_gather(
                                    er_g[:, b0:b0 + nn, :], er_tbl[:],
                                    idxe_t[:, b0 * 8:(b0 + nn) * 8],
                                    nn * P, nn * P, ER_ELEM)
                            e_f = work.tile([P, maxch, H], f32, tag="ef")
                            if nlo:
                                nc.vector.tensor_tensor(
                                    out=e_f[:, :nlo, :], in0=g_lo[:, :nlo, HD:HD + H],
                                    in1=er_g[:, :nlo, 0:H], op=mybir.AluOpType.add)
                            if nhi:
                                nc.vector.tensor_tensor(
                                    out=e_f[:, nlo:nchk, :], in0=g_hi[:, :nhi, HD:HD + H],
                                    in1=er_g[:, nlo:nchk, 0:H], op=mybir.AluOpType.add)
                            nc.vector.tensor_tensor(
                                out=e_f[:, :nchk, :], in0=e_f[:, :nchk, :],
                                in1=maskt_s[:, c0:c0 + nchk].unsqueeze(2).to_broadcast([P, nchk, H]),
                                op=mybir.AluOpType.add)
                            lk = work.tile([P, maxch, H], f32, tag="lk")
                            nc.vector.tensor_scalar_mul(lk[:, :nchk, :], e_f[:, :nchk, :], NEG_SLOPE)
                            nc.vector.tensor_tensor(
                                out=lk[:, :nchk, :], in0=lk[:, :nchk, :],
                                in1=e_f[:, :nchk, :], op=mybir.AluOpType.max)
                            nc.scalar.activation(
                                out=w_all[:, c0:c0 + nchk, :], in_=lk[:, :nchk, :],
                                func=mybir.ActivationFunctionType.Exp)
                        # weight rows in place; hop1 also writes w into col 256:260
                        for g_t, a, b in (((g_lo, 0, nlo), (g_hi, nlo, nchk)) if not no_w else ()):
                            nn = b - a
                            if nn == 0:
                                continue
                            nc.vector.tensor_tensor(
                                out=g_t[:, :nn, 0:HD].rearrange("p c (h d) -> p c h d", h=H),
                                in0=g_t[:, :nn, 0:HD].rearrange("p c (h d) -> p c h d", h=H),
                                in1=w_all[:, c0 + a:c0 + b, :].unsqueeze(3).to_broadcast([P, nn, H, D]),
                                op=mybir.AluOpType.mult)
                            if k == 1:
                                nc.vector.tensor_copy(
                                    out=g_t[:, :nn, HD:HD + H],
                                    in_=w_all[:, c0 + a:c0 + b, :])
                        for i, (qoff, hf) in enumerate(tl["chunks"] if not no_mm else []):
                            g_t = g_lo if i < nlo else g_hi
                            ii = i if i < nlo else i - nlo
                            nc.tensor.matmul(
                                ps[qoff:qoff + SUBW, 0:ncol],
                                lhsT=sel_all[:, c0 + i, :],
                                rhs=g_t[:, ii, 0:ncol],
                                start=False, stop=(i == nchk - 1),
                            )
                    # drain
                    if k == 1:
                        s_eps = work.tile([P, H], f32, tag="seps")
                        nc.vector.tensor_scalar_add(s_eps[:], ps[:, HD:HD + H], 1e-30)
                        nc.vector.reciprocal(out=recip_all[:, t, :], in_=s_eps[:])
                    hk = work.tile([P, HD], bf16, tag="hk")
                    nc.vector.tensor_tensor(
                        out=hk[:].rearrange("p (h d) -> p h d", h=H),
                        in0=ps[:, 0:HD].rearrange("p (h d) -> p h d", h=H),
                        in1=recip_all[:, t, :].unsqueeze(2).to_broadcast([P, H, D]),
                        op=mybir.AluOpType.mult)
                    if k < K:
                        sh, r0 = (shardA[k], t * P) if t < NT_A else (shardB[k], t * P - HALF)
                        nc.sync.dma_start(out=sh[r0:r0 + P, :], in_=hk[:])
                    hs = work.tile([P, HD], bf16, tag="hs")
                    nc.vector.tensor_tensor(out=hs[:], in0=hk[:], in1=pos_s[:, k, :],
                                            op=mybir.AluOpType.add)
                    nc.sync.dma_start(out=hstack[t * P:(t + 1) * P, k, :], in_=hs[:])
                    if debug:
                        nc.sync.dma_start(out=dbg_hs[t * P:(t + 1) * P, k, :], in_=hs[:])
                    if k < K:
                        if t == NT_A - 1:
                            nc.gpsimd.collective_compute(
                                "AllGather", mybir.AluOpType.bypass, replica_groups=rg,
                                ins=[shardA[k].opt()], outs=[tblA[k].opt()])
                        elif t == NT - 1:
                            nc.gpsimd.collective_compute(
                                "AllGather", mybir.AluOpType.bypass, replica_groups=rg,
                                ins=[shardB[k].opt()], outs=[tblB[k].opt()])

            max_hop = int(os.environ.get("AGDN_MAX_HOP", str(K)))
            no_mm = bool(int(os.environ.get("AGDN_NO_MM", "0")))
            no_w = bool(int(os.environ.get("AGDN_NO_W", "0")))
            no_gat = bool(int(os.environ.get("AGDN_NO_GAT", "0")))
            do_final = bool(int(os.environ.get("AGDN_FINAL", "1")))
            for k in range(1, max_hop + 1):
                hop(k)

            if not do_final:
                for t in range(NT):
                    z = work.tile([P, HD], f32, tag="rst")
                    nc.vector.memset(z[:], 0.0)
                    nc.sync.dma_start(out=out_ext[t * P:(t + 1) * P, :], in_=z[:])
            if debug:
                nc.sync.dma_start(out=dbg_w[:, :min(NCHUNK, 1024), :],
                                  in_=w_all[:, :min(NCHUNK, 1024), :])
                nc.sync.dma_start(out=dbg_recip[:], in_=recip_all[:])
            # ---- final: hop attention ----
            for t in range(NT if do_final else 0):
                hst = work.tile([P, K + 1, HD], bf16, tag="hst")
                nc.sync.dma_start(out=hst[:], in_=hstack[t * P:(t + 1) * P, :, :])
                prod = work.tile([P, (K + 1) * HD], f32, tag="prod")
                nc.vector.tensor_tensor(
                    out=prod[:].rearrange("p (k e) -> p k e", k=K + 1),
                    in0=hst[:], in1=hopl_s[:].unsqueeze(1).to_broadcast([P, K + 1, HD]),
                    op=mybir.AluOpType.mult)
                a_l = work.tile([P, (K + 1) * H], f32, tag="al")
                nc.vector.tensor_reduce(
                    out=a_l[:], in_=prod[:].rearrange("p (k h d) -> p k h d", k=K + 1, h=H),
                    axis=mybir.AxisListType.X, op=mybir.AluOpType.add)
                prr = work.tile([P, HD], f32, tag="prr")
                nc.vector.tensor_tensor(out=prr[:], in0=hst[:, 0, :], in1=hopr_s[:],
                                        op=mybir.AluOpType.mult)
                a_r = work.tile([P, H], f32, tag="ar")
                nc.vector.tensor_reduce(
                    out=a_r[:], in_=prr[:].rearrange("p (h d) -> p h d", h=H),
                    axis=mybir.AxisListType.X, op=mybir.AluOpType.add)
                # logits (transposed to [h, k] layout on write)
                lg = work.tile([P, H * (K + 1)], f32, tag="lg")   # [h, k]-major
                lg_kh = lg[:].rearrange("p (h k) -> p k h", h=H, k=K + 1)
                nc.vector.tensor_tensor(
                    out=lg_kh, in0=a_l[:].rearrange("p (k h) -> p k h", k=K + 1),
                    in1=a_r[:].unsqueeze(1).to_broadcast([P, K + 1, H]),
                    op=mybir.AluOpType.add)
                lk2 = work.tile([P, H * (K + 1)], f32, tag="lk2")
                nc.vector.tensor_scalar_mul(lk2[:], lg[:], NEG_SLOPE)
                nc.vector.tensor_tensor(out=lk2[:], in0=lk2[:], in1=lg[:],
                                        op=mybir.AluOpType.max)
                ex = work.tile([P, H * (K + 1)], f32, tag="ex")
                nc.scalar.activation(out=ex[:], in_=lk2[:],
                                     func=mybir.ActivationFunctionType.Exp)
                den = work.tile([P, H], f32, tag="den")
                nc.vector.tensor_reduce(
                    out=den[:], in_=ex[:].rearrange("p (h k) -> p h k", h=H),
                    axis=mybir.AxisListType.X, op=mybir.AluOpType.add)
                rden = work.tile([P, H], f32, tag="rden")
                nc.vector.reciprocal(out=rden[:], in_=den[:])
                hopa = work.tile([P, H * (K + 1)], f32, tag="hopa")
                nc.vector.tensor_tensor(
                    out=hopa[:].rearrange("p (h k) -> p h k", h=H),
                    in0=ex[:].rearrange("p (h k) -> p h k", h=H),
                    in1=rden[:].unsqueeze(2).to_broadcast([P, H, K + 1]),
                    op=mybir.AluOpType.mult)
                # rst = sum_k hopa * hs  (prod2 written [h, d, k]-major)
                pr2 = work.tile([P, HD * (K + 1)], f32, tag="pr2")
                nc.vector.tensor_tensor(
                    out=pr2[:].rearrange("p (h d k) -> p k h d", h=H, d=D, k=K + 1),
                    in0=hst[:].rearrange("p k (h d) -> p k h d", h=H),
                    in1=hopa[:].rearrange("p (h k) -> p k h", h=H).unsqueeze(3)
                        .to_broadcast([P, K + 1, H, D]),
                    op=mybir.AluOpType.mult)
                rst = work.tile([P, HD], f32, tag="rst")
                nc.vector.tensor_reduce(
                    out=rst[:], in_=pr2[:].rearrange("p (e k) -> p e k", k=K + 1),
                    axis=mybir.AxisListType.X, op=mybir.AluOpType.add)
                nc.vector.tensor_tensor(out=rst[:], in0=rst[:], in1=bias_s[:],
                                        op=mybir.AluOpType.add)
                nc.sync.dma_start(out=out_ext[t * P:(t + 1) * P, :], in_=rst[:])

    nc.compile()
    return nc


# ---------------- entry point ----------------

def kernel(**inputs) -> np.ndarray:
    in_maps, tiles, NSLOT, NCHUNK = _prep(**inputs)
    nc = _build(tiles, NSLOT, NCHUNK)
    trace = bool(int(os.environ.get("AGDN_TRACE", "0")))
    res = run_bass_kernel_spmd(nc, in_maps, core_ids=list(range(NC)), trace=trace,
                               tmpdir=os.environ.get("AGDN_TMPDIR") or None)
    if trace:
        kernel.last_exec_time_ns = res.exec_time_ns
        kernel.last_res = res
    out = np.empty((N, H, D), np.float32)
    for c in range(NC):
        o = res.results[c]["out"]                    # [NLOCP, HD]
        out[c * NLOC:c * NLOC + HALF_DATA] = o[:HALF_DATA].reshape(-1, H, D)
        out[c * NLOC + HALF_DATA:(c + 1) * NLOC] = \
            o[HALF:HALF + HALF_DATA].reshape(-1, H, D)
    return out



# revision 13
# speedup vs baseline: 1.5029x; 1.5029x over previous
"""AGDNConv (GAT transition + K-hop diffusion + hop attention) on 8 TRN2 NeuronCores.

Strategy (dst-sharded graph parallel), v2 — gather-descriptor-bound redesign:
- Nodes sharded contiguously: core c owns nodes [c*6250, (c+1)*6250).
- Stage 1 (per core): fc matmul (H0^T = W @ feat^T), attention projections
  el/er; hop-1 table rows are [H0 | el | pad] bf16 (768B); er stays in SBUF.
- Per-hop node tables (bf16) are replicated via AllGather in two halves
  (A/B by local index) so hop k+1's half-A gathers overlap half-B's collective.
- Edge slots are grouped (half, dst-tile): per tile, chunks of 128 slots.
  The SWDGE dma_gather descriptor generation on GpSimd (~5.2ns/idx + ~2us/call)
  is the kernel bottleneck, so gathers are batched: one call per (half x
  3-tile batch), ~34 calls/hop, ~112k idx/hop.
- Aggregation: per 128-slot chunk, one PE matmul with a host-precomputed
  128-wide 0/1 selection matrix (fp8, DMA'd from DRAM) accumulating into the
  dst tile's PSUM. Pad slots have all-zero sel columns (no masking needed).
- Per-edge softmax weights w = exp(leaky(el[src]+er[dst])): el comes from the
  hop-1 gathered rows; er[dst] per slot is produced by a tiny PE matmul with
  the TRANSPOSED selection matrix (selT, also host-precomputed) against the
  tile's er values resident in SBUF -- this replaces the baseline's 1ms
  per-edge er gather. Softmax denominators ride as 4 extra w columns in the
  hop-1 aggregation matmul; every hop's PSUM is scaled by 1/s at drain.
- The hop-attention finale is fused into hop-3's tile loop (batched), so no
  serial tail.

All index manipulation (sharding, slot layout, sel matrices) happens on the
host; all floating-point math runs on device.
"""

import os
import numpy as np
import ml_dtypes

import concourse.bacc as bacc
import concourse.bass as bass
import concourse.mybir as mybir
import concourse.tile as tile
from concourse.bass_utils import run_bass_kernel_spmd
from concourse.masks import make_identity

# ---------------- problem geometry (hardcoded per spec) ----------------
N, E, IN, H, D, K = 50000, 800000, 256, 4, 64, 3
HD = H * D                     # 256
NC = 8
NLOC = N // NC                 # 6250
HALF_DATA = NLOC // 2          # 3125
HALF = 3200                    # padded rows per half (25 tiles of 128)
NLOCP = 2 * HALF               # 6400
NT = NLOCP // 128              # 50 dst tiles
NT_A = HALF // 128             # 25 (tiles 0..24 are half A)
TROWS = NC * HALF              # 25600 rows per half-table (int16-safe)
ELEM1 = 384                    # hop-1 table row elems (256 H0 + 4 el + pad)
ELEM = 256
NEG_SLOPE = 0.2
TB = 2                         # dst tiles per gather/compute batch

P = 128
f32 = mybir.dt.float32
bf16 = mybir.dt.bfloat16
fp8 = mybir.dt.float8e4
i16 = mybir.dt.int16


# ---------------- host-side preparation ----------------

def _prep(feat, src, dst, W_src, attn_l, attn_r, hop_attn_l, hop_attn_r,
          pos_emb, bias):
    src = np.asarray(src).astype(np.int64)
    dst = np.asarray(dst).astype(np.int64)
    feat = np.asarray(feat, dtype=np.float32)

    core = dst // NLOC
    j = dst % NLOC
    jp = j + (HALF - HALF_DATA) * (j >= HALF_DATA)  # padded local coord of dst
    t_of = jp >> 7
    dl = jp & 127

    sc = src // NLOC
    sj = src % NLOC
    shalf = (sj >= HALF_DATA).astype(np.int64)
    tblidx = sc * HALF + sj - HALF_DATA * shalf     # row within half-table

    # group = (src-half, dst-tile); slot space = [A:t0..t49 | B:t0..t49]
    NG = 2 * NT
    g_of = shalf * NT + t_of

    counts = np.zeros((NC, NG), np.int64)
    for c in range(NC):
        counts[c] = np.bincount(g_of[core == c], minlength=NG)
    nch_g = (np.max(counts, axis=0) + 127) // 128          # chunks per group
    cap_g = nch_g * 128
    gbase = np.zeros(NG + 1, np.int64)
    np.cumsum(cap_g, out=gbase[1:])
    NSLOT = int(gbase[-1])
    NCHUNK = NSLOT // 128

    # per-core slot arrays
    idx_main = np.zeros((NC, NSLOT), np.int16)   # pad slots read row 0 (killed by zero sel col)
    sel_in = np.zeros((NC, P, NCHUNK * 128), np.uint8)
    selT_in = np.zeros((NC, P, NCHUNK * 128), np.uint8)
    one_fp8 = np.array(1.0, dtype=ml_dtypes.float8_e4m3).view(np.uint8)

    for c in range(NC):
        m = np.nonzero(core == c)[0]
        o = m[np.argsort(g_of[m], kind="stable")]
        g_sorted = g_of[o]
        gstart = np.zeros(NG + 1, np.int64)
        np.cumsum(counts[c], out=gstart[1:])
        rank = np.arange(len(o)) - gstart[g_sorted]
        slot = gbase[g_sorted] + rank
        idx_main[c, slot] = tblidx[o]
        ch = slot >> 7
        sp = slot & 127
        d_ = dl[o]
        sel_in[c, sp, ch * 128 + d_] = one_fp8
        selT_in[c, d_, ch * 128 + sp] = one_fp8

    def wrap16(a):
        w = a.reshape(NSLOT // 16, 16).T            # [16, NSLOT//16]
        return np.tile(w, (8, 1)).copy()            # [128, NSLOT//16]

    # device-side schedule: per tile, chunk offsets; per batch, gather ranges
    tiles = []
    for t in range(NT):
        tiles.append({
            "cA0": int(gbase[t] // 128), "nA": int(nch_g[t]),
            "cB0": int(gbase[NT + t] // 128), "nB": int(nch_g[NT + t]),
        })
    batches = []
    for b0 in range(0, NT, TB):
        ts = list(range(b0, min(b0 + TB, NT)))
        cA0 = tiles[ts[0]]["cA0"]
        nnA = sum(tiles[t]["nA"] for t in ts)
        cB0 = tiles[ts[0]]["cB0"]
        nnB = sum(tiles[t]["nB"] for t in ts)
        batches.append({"tiles": ts, "cA0": cA0, "nnA": nnA,
                        "cB0": cB0, "nnB": nnB})
    NNMAX = max(b["nnA"] + b["nnB"] for b in batches)

    # feat^T padded per core
    featT = np.zeros((NC, IN, NLOCP), np.float32)
    for c in range(NC):
        fc = feat[c * NLOC:(c + 1) * NLOC]          # [6250, 256]
        featT[c, :, :HALF_DATA] = fc[:HALF_DATA].T
        featT[c, :, HALF:HALF + HALF_DATA] = fc[HALF_DATA:].T

    WT = np.ascontiguousarray(np.asarray(W_src, np.float32).T)    # [IN, HD]
    al = np.asarray(attn_l, np.float32).reshape(H, D)
    ar = np.asarray(attn_r, np.float32).reshape(H, D)
    AlAr = np.zeros((HD, 2 * H), np.float32)
    for h in range(H):
        AlAr[h * D:(h + 1) * D, h] = al[h]
        AlAr[h * D:(h + 1) * D, H + h] = ar[h]

    hopl = np.asarray(hop_attn_l, np.float32).reshape(H * D)
    hopr = np.asarray(hop_attn_r, np.float32).reshape(H * D)
    hopl_r = np.broadcast_to(hopl, (P, HD)).copy()
    hopr_r = np.broadcast_to(hopr, (P, HD)).copy()
    pos = np.asarray(pos_emb, np.float32).reshape(H, K + 1, D)
    pos_flat = np.transpose(pos, (1, 0, 2)).reshape(K + 1, HD)    # [k, h*64+d]
    pos_r = np.broadcast_to(pos_flat, (P, K + 1, HD)).copy()
    bias_r = np.broadcast_to(np.asarray(bias, np.float32).reshape(HD), (P, HD)).copy()

    in_maps = []
    for c in range(NC):
        in_maps.append({
            "featT": featT[c],
            "WT": WT,
            "AlAr": AlAr,
            "hopl": hopl_r,
            "hopr": hopr_r,
            "pos": pos_r,
            "bias": bias_r,
            "idxm": wrap16(idx_main[c]),
            "selin": sel_in[c].view(ml_dtypes.float8_e4m3),
            "selTin": selT_in[c].view(ml_dtypes.float8_e4m3),
        })
    return in_maps, tiles, batches, NSLOT, NCHUNK, NNMAX


# ---------------- device kernel ----------------

def _build(tiles, batches, NSLOT, NCHUNK, NNMAX):
    nc = bacc.Bacc("TRN2", debug=False)

    featT_in = nc.dram_tensor("featT", [IN, NLOCP], f32, kind="ExternalInput")
    WT_in = nc.dram_tensor("WT", [IN, HD], f32, kind="ExternalInput")
    AlAr_in = nc.dram_tensor("AlAr", [HD, 2 * H], f32, kind="ExternalInput")
    hopl_in = nc.dram_tensor("hopl", [P, HD], f32, kind="ExternalInput")
    hopr_in = nc.dram_tensor("hopr", [P, HD], f32, kind="ExternalInput")
    pos_in = nc.dram_tensor("pos", [P, K + 1, HD], f32, kind="ExternalInput")
    bias_in = nc.dram_tensor("bias", [P, HD], f32, kind="ExternalInput")
    idxm_in = nc.dram_tensor("idxm", [P, NSLOT // 16], i16, kind="ExternalInput")
    selin_in = nc.dram_tensor("selin", [P, NCHUNK * 128], fp8, kind="ExternalInput")
    selTin_in = nc.dram_tensor("selTin", [P, NCHUNK * 128], fp8, kind="ExternalInput")
    out_ext = nc.dram_tensor("out", [NLOCP, HD], f32, kind="ExternalOutput")

    rg = [list(range(NC))]

    with tile.TileContext(nc) as tc:
        with (
            tc.tile_pool(name="dram", bufs=1, space="DRAM") as dram,
            tc.tile_pool(name="pers", bufs=1) as pers,
            tc.tile_pool(name="work", bufs=2) as work,
            tc.tile_pool(name="gat", bufs=2) as gat,
            tc.tile_pool(name="fin", bufs=2) as finp,
            tc.tile_pool(name="psum", bufs=2, space="PSUM") as psum,
            tc.tile_pool(name="apsum", bufs=2, space="PSUM") as apsum,
            tc.tile_pool(name="epsum", bufs=2, space="PSUM") as epsum,
        ):
            # ---- persistent DRAM ----
            shardA = [dram.tile([HALF, ELEM1 if k == 1 else ELEM], bf16,
                                tag=f"shA{k}", name=f"shA{k}") for k in (1, 2, 3)]
            shardB = [dram.tile([HALF, ELEM1 if k == 1 else ELEM], bf16,
                                tag=f"shB{k}", name=f"shB{k}") for k in (1, 2, 3)]
            tblA = [dram.tile([TROWS, ELEM1 if k == 1 else ELEM], bf16, addr_space="Shared",
                              tag=f"tbA{k}", name=f"tbA{k}") for k in (1, 2, 3)]
            tblB = [dram.tile([TROWS, ELEM1 if k == 1 else ELEM], bf16, addr_space="Shared",
                              tag=f"tbB{k}", name=f"tbB{k}") for k in (1, 2, 3)]
            hstack = dram.tile([NLOCP, K, HD], bf16, tag="hstack")

            # ---- persistent SBUF ----
            w_all = pers.tile([P, NCHUNK, H], bf16, tag="w")
            recip_all = pers.tile([P, NT, H], f32, tag="recip")
            er_all = pers.tile([P, NT, H], bf16, tag="erall")
            idxm = pers.tile([P, NSLOT // 16], i16, tag="idxm")
            hopl_s = pers.tile([P, HD], f32, tag="hopl")
            hopr_s = pers.tile([P, HD], f32, tag="hopr")
            pos_s = pers.tile([P, K + 1, HD], f32, tag="pos")
            bias_s = pers.tile([P, HD], f32, tag="bias")
            ident = pers.tile([P, P], bf16, tag="ident")

            nc.sync.dma_start(out=idxm[:], in_=idxm_in[:])
            nc.sync.dma_start(out=hopl_s[:], in_=hopl_in[:])
            nc.sync.dma_start(out=hopr_s[:], in_=hopr_in[:])
            nc.sync.dma_start(out=pos_s[:], in_=pos_in[:])
            nc.sync.dma_start(out=bias_s[:], in_=bias_in[:])
            make_identity(nc, ident[:])

            # ---- stage 1: fc + el/er + hop-1 table + hstack0 ----
            WT_s = pers.tile([P, 2, HD], f32, tag="wts")     # [k-chunk, o]
            AlAr_s = pers.tile([P, 2, 2 * H], f32, tag="alar")
            nc.sync.dma_start(out=WT_s[:], in_=WT_in[:].rearrange("(a p) o -> p a o", p=P))
            nc.sync.dma_start(out=AlAr_s[:], in_=AlAr_in[:].rearrange("(a p) o -> p a o", p=P))
            AlAr_bf = pers.tile([P, 2, 2 * H], bf16, tag="alarbf")
            nc.vector.tensor_copy(out=AlAr_bf[:], in_=AlAr_s[:])

            NB = 512

            def stage1_block(nb):
                n0 = nb * NB
                w_ = min(NB, NLOCP - n0)
                ft = [work.tile([P, NB], f32, tag=f"ft{i}", name=f"ft{i}") for i in range(2)]
                for kc in range(2):
                    nc.sync.dma_start(out=ft[kc][:, :w_],
                                      in_=featT_in[kc * P:(kc + 1) * P, n0:n0 + w_])
                h0t_sb = work.tile([P, 2, NB], bf16, tag="h0t")
                for oh in range(2):
                    h0t_ps = psum.tile([P, NB], f32, space="PSUM", tag="h0tp")
                    for kc in range(2):
                        nc.tensor.matmul(
                            h0t_ps[:, :w_],
                            lhsT=WT_s[:, kc, oh * P:(oh + 1) * P],
                            rhs=ft[kc][:, :w_],
                            start=(kc == 0), stop=(kc == 1),
                        )
                    nc.vector.tensor_copy(out=h0t_sb[:, oh, :w_], in_=h0t_ps[:, :w_])
                for sub in range(w_ // P):
                    t = (n0 + sub * P) // P
                    eler_ps = psum.tile([P, 2 * H], f32, space="PSUM", tag="elerp", bufs=1)
                    for oh in range(2):
                        nc.tensor.matmul(
                            eler_ps[:],
                            lhsT=h0t_sb[:, oh, sub * P:(sub + 1) * P],
                            rhs=AlAr_bf[:, oh, :],
                            start=(oh == 0), stop=(oh == 1),
                        )
                    h0row_ps = psum.tile([P, HD], bf16, space="PSUM", tag="h0rp", bufs=1)
                    for oh in range(2):
                        nc.tensor.transpose(
                            out=h0row_ps[:, oh * P:(oh + 1) * P],
                            in_=h0t_sb[:, oh, sub * P:(sub + 1) * P],
                            identity=ident[:],
                        )
                    row_sb = work.tile([P, ELEM1], bf16, tag="row1")
                    nc.vector.tensor_copy(out=row_sb[:, 0:HD], in_=h0row_ps[:])
                    nc.vector.tensor_copy(out=row_sb[:, HD:HD + H], in_=eler_ps[:, 0:H])
                    nc.vector.memset(row_sb[:, HD + H:], 0.0)
                    sh, r0 = (shardA[0], t * P) if t < NT_A else (shardB[0], (t - NT_A) * P)
                    nc.sync.dma_start(out=sh[r0:r0 + P, :], in_=row_sb[:])
                    # er for this tile stays resident in SBUF
                    nc.vector.tensor_copy(out=er_all[:, t, :], in_=eler_ps[:, H:2 * H])
                    # hstack k=0: H0 + pos[0]
                    hs0 = work.tile([P, HD], bf16, tag="hs")
                    nc.vector.tensor_tensor(out=hs0[:], in0=h0row_ps[:],
                                            in1=pos_s[:, 0, :], op=mybir.AluOpType.add)
                    nc.sync.dma_start(out=hstack[t * P:(t + 1) * P, 0, :], in_=hs0[:])

            for nb in range(NLOCP // NB + (1 if NLOCP % NB else 0)):
                stage1_block(nb)
                if nb * NB < HALF <= (nb + 1) * NB:
                    nc.gpsimd.collective_compute(
                        "AllGather", mybir.AluOpType.bypass, replica_groups=rg,
                        ins=[shardA[0].opt()], outs=[tblA[0].opt()])
            nc.gpsimd.collective_compute(
                "AllGather", mybir.AluOpType.bypass, replica_groups=rg,
                ins=[shardB[0].opt()], outs=[tblB[0].opt()])

            # ---- hops ----
            def hop(k):
                """k = 1,2,3: read tbl[k-1], write shard[k]/tbl[k] (k<3), hstack k.
                k==K folds the hop-attention finale into the tile loop."""
                el1 = ELEM1 if k == 1 else ELEM
                ncol = HD + H if k == 1 else HD
                tA, tB = tblA[k - 1], tblB[k - 1]
                for b in batches:
                    ts = b["tiles"]
                    cA0, nnA, cB0, nnB = b["cA0"], b["nnA"], b["cB0"], b["nnB"]
                    nn = nnA + nnB
                    g_flat = gat.tile([P, NNMAX * ELEM1], bf16, tag="g")
                    g = g_flat[:, 0:NNMAX * el1].rearrange("p (c s) -> p c s", s=el1)
                    gcap = int(os.environ.get("AGDN_GCAP", "8"))
                    for tbl_, c0_, o0_, nn_ in ((tA, cA0, 0, nnA),
                                                (tB, cB0, nnA, nnB)):
                        for b0_ in range(0, nn_, gcap):
                            n_ = min(gcap, nn_ - b0_)
                            nc.gpsimd.dma_gather(
                                g[:, o0_ + b0_:o0_ + b0_ + n_, :], tbl_[:],
                                idxm[:, (c0_ + b0_) * 8:(c0_ + b0_ + n_) * 8],
                                n_ * P, n_ * P, el1)
                    selb = gat.tile([P, NNMAX, 128], fp8, tag="selb")
                    nc.sync.dma_start(out=selb[:, 0:nnA, :],
                                      in_=selin_in[:, cA0 * 128:(cA0 + nnA) * 128]
                                      .rearrange("p (c s) -> p c s", s=128))
                    nc.sync.dma_start(out=selb[:, nnA:nn, :],
                                      in_=selin_in[:, cB0 * 128:(cB0 + nnB) * 128]
                                      .rearrange("p (c s) -> p c s", s=128))

                    # local chunk list per tile: (local idx in g/selb, tile)
                    loc_of_tile = {}
                    off = 0
                    for t in ts:
                        loc_of_tile[t] = {"A": (off, tiles[t]["nA"])}
                        off += tiles[t]["nA"]
                    for t in ts:
                        loc_of_tile[t]["B"] = (off, tiles[t]["nB"])
                        off += tiles[t]["nB"]

                    if k == 1:
                        selTb = gat.tile([P, NNMAX, 128], fp8, tag="selTb")
                        nc.sync.dma_start(out=selTb[:, 0:nnA, :],
                                          in_=selTin_in[:, cA0 * 128:(cA0 + nnA) * 128]
                                          .rearrange("p (c s) -> p c s", s=128))
                        nc.sync.dma_start(out=selTb[:, nnA:nn, :],
                                          in_=selTin_in[:, cB0 * 128:(cB0 + nnB) * 128]
                                          .rearrange("p (c s) -> p c s", s=128))
                        # er per slot via selT matmuls: er_ps[:, i*H:(i+1)*H]
                        er_ps = epsum.tile([P, NNMAX * H], f32, space="PSUM", tag="erps")
                        for t in ts:
                            for piece in ("A", "B"):
                                o0, cnt = loc_of_tile[t][piece]
                                for i in range(o0, o0 + cnt):
                                    nc.tensor.matmul(
                                        er_ps[:, i * H:(i + 1) * H],
                                        lhsT=selTb[:, i, :],
                                        rhs=er_all[:, t, :],
                                        start=True, stop=True)
                        # e = el + er ; w = exp(max(e, .2e))
                        e_f = work.tile([P, NNMAX, H], f32, tag="ef")
                        nc.vector.tensor_tensor(
                            out=e_f[:, :nn, :], in0=g[:, :nn, HD:HD + H],
                            in1=er_ps[:].rearrange("p (c h) -> p c h", h=H)[:, :nn, :],
                            op=mybir.AluOpType.add)
                        lk = work.tile([P, NNMAX, H], f32, tag="lk")
                        nc.vector.tensor_scalar_mul(lk[:, :nn, :], e_f[:, :nn, :], NEG_SLOPE)
                        nc.vector.tensor_tensor(
                            out=lk[:, :nn, :], in0=lk[:, :nn, :],
                            in1=e_f[:, :nn, :], op=mybir.AluOpType.max)
                        wb = work.tile([P, NNMAX, H], f32, tag="wb")
                        nc.scalar.activation(
                            out=wb[:, :nn, :], in_=lk[:, :nn, :],
                            func=mybir.ActivationFunctionType.Exp)
                        # persist w (bf16) for hops 2,3 at global chunk coords
                        nc.vector.tensor_copy(out=w_all[:, cA0:cA0 + nnA, :],
                                              in_=wb[:, 0:nnA, :])
                        nc.vector.tensor_copy(out=w_all[:, cB0:cB0 + nnB, :],
                                              in_=wb[:, nnA:nn, :])
                        # weight rows; write w into cols 256:260 for denominators
                        nc.vector.tensor_tensor(
                            out=g[:, :nn, 0:HD].rearrange("p c (h d) -> p c h d", h=H),
                            in0=g[:, :nn, 0:HD].rearrange("p c (h d) -> p c h d", h=H),
                            in1=wb[:, :nn, :].unsqueeze(3).to_broadcast([P, nn, H, D]),
                            op=mybir.AluOpType.mult)
                        nc.vector.tensor_copy(out=g[:, :nn, HD:HD + H], in_=wb[:, :nn, :])
                    else:
                        nc.vector.tensor_tensor(
                            out=g[:, 0:nnA, 0:HD].rearrange("p c (h d) -> p c h d", h=H),
                            in0=g[:, 0:nnA, 0:HD].rearrange("p c (h d) -> p c h d", h=H),
                            in1=w_all[:, cA0:cA0 + nnA, :].unsqueeze(3)
                                .to_broadcast([P, nnA, H, D]),
                            op=mybir.AluOpType.mult)
                        nc.vector.tensor_tensor(
                            out=g[:, nnA:nn, 0:HD].rearrange("p c (h d) -> p c h d", h=H),
                            in0=g[:, nnA:nn, 0:HD].rearrange("p c (h d) -> p c h d", h=H),
                            in1=w_all[:, cB0:cB0 + nnB, :].unsqueeze(3)
                                .to_broadcast([P, nnB, H, D]),
                            op=mybir.AluOpType.mult)

                    if k == K:
                        hst = finp.tile([P, TB, K + 1, HD], bf16, tag="hst")
                        for tl, t in enumerate(ts):
                            nc.sync.dma_start(
                                out=hst[:, tl, 0:K, :],
                                in_=hstack[t * P:(t + 1) * P, :, :])

                    # aggregation matmuls + drain per tile
                    for tl, t in enumerate(ts):
                        chunks = []
                        for piece in ("A", "B"):
                            o0, cnt = loc_of_tile[t][piece]
                            chunks.extend(range(o0, o0 + cnt))
                        ps = apsum.tile([P, 320], f32, space="PSUM", tag="agg")
                        for ci, i in enumerate(chunks):
                            nc.tensor.matmul(
                                ps[:, 0:ncol],
                                lhsT=selb[:, i, :],
                                rhs=g[:, i, 0:ncol],
                                start=(ci == 0), stop=(ci == len(chunks) - 1),
                            )
                        if k == 1:
                            s_eps = work.tile([P, H], f32, tag="seps")
                            nc.vector.tensor_scalar_add(s_eps[:], ps[:, HD:HD + H], 1e-30)
                            nc.vector.reciprocal(out=recip_all[:, t, :], in_=s_eps[:])
                        if k < K:
                            hk = work.tile([P, HD], bf16, tag="hk")
                            nc.vector.tensor_tensor(
                                out=hk[:].rearrange("p (h d) -> p h d", h=H),
                                in0=ps[:, 0:HD].rearrange("p (h d) -> p h d", h=H),
                                in1=recip_all[:, t, :].unsqueeze(2).to_broadcast([P, H, D]),
                                op=mybir.AluOpType.mult)
                            sh, r0 = (shardA[k], t * P) if t < NT_A else \
                                     (shardB[k], (t - NT_A) * P)
                            nc.sync.dma_start(out=sh[r0:r0 + P, :], in_=hk[:])
                            hs = work.tile([P, HD], bf16, tag="hs2")
                            nc.vector.tensor_tensor(out=hs[:], in0=hk[:],
                                                    in1=pos_s[:, k, :],
                                                    op=mybir.AluOpType.add)
                            nc.sync.dma_start(out=hstack[t * P:(t + 1) * P, k, :], in_=hs[:])
                        else:
                            # k==K: write h3+pos3 straight into the finale buffer
                            tmp = work.tile([P, HD], f32, tag="hk3f")
                            nc.vector.tensor_tensor(
                                out=tmp[:].rearrange("p (h d) -> p h d", h=H),
                                in0=ps[:, 0:HD].rearrange("p (h d) -> p h d", h=H),
                                in1=recip_all[:, t, :].unsqueeze(2).to_broadcast([P, H, D]),
                                op=mybir.AluOpType.mult)
                            nc.vector.tensor_tensor(out=hst[:, tl, K, :], in0=tmp[:],
                                                    in1=pos_s[:, K, :],
                                                    op=mybir.AluOpType.add)
                        if k < K:
                            if t == NT_A - 1:
                                nc.gpsimd.collective_compute(
                                    "AllGather", mybir.AluOpType.bypass, replica_groups=rg,
                                    ins=[shardA[k].opt()], outs=[tblA[k].opt()])
                            elif t == NT - 1:
                                nc.gpsimd.collective_compute(
                                    "AllGather", mybir.AluOpType.bypass, replica_groups=rg,
                                    ins=[shardB[k].opt()], outs=[tblB[k].opt()])

                    # ---- fused hop-attention finale for this batch ----
                    if k == K:
                        nt_ = len(ts)
                        KP = K + 1
                        prod = finp.tile([P, TB * KP * HD], f32, tag="prod", bufs=1)
                        nc.vector.tensor_tensor(
                            out=prod[:].rearrange("p (t k e) -> p t k e", t=TB, k=KP)[:, :nt_],
                            in0=hst[:, :nt_],
                            in1=hopl_s[:].unsqueeze(1).unsqueeze(1)
                                .to_broadcast([P, nt_, KP, HD]),
                            op=mybir.AluOpType.mult)
                        a_l = finp.tile([P, TB * KP * H], f32, tag="al", bufs=1)
                        for tl in range(nt_):
                            nc.vector.tensor_reduce(
                                out=a_l[:].rearrange("p (t k h) -> p t k h",
                                                     t=TB, k=KP)[:, tl],
                                in_=prod[:].rearrange("p (t k h d) -> p t k h d",
                                                      t=TB, k=KP, h=H)[:, tl],
                                axis=mybir.AxisListType.X, op=mybir.AluOpType.add)
                        prr = finp.tile([P, TB * HD], f32, tag="prr", bufs=1)
                        nc.vector.tensor_tensor(
                            out=prr[:].rearrange("p (t e) -> p t e", t=TB)[:, :nt_],
                            in0=hst[:, :nt_, 0, :],
                            in1=hopr_s[:].unsqueeze(1).to_broadcast([P, nt_, HD]),
                            op=mybir.AluOpType.mult)
                        a_r = finp.tile([P, TB * H], f32, tag="ar", bufs=1)
                        nc.vector.tensor_reduce(
                            out=a_r[:].rearrange("p (t h) -> p t h", t=TB)[:, :nt_],
                            in_=prr[:].rearrange("p (t h d) -> p t h d", t=TB, h=H)[:, :nt_],
                            axis=mybir.AxisListType.X, op=mybir.AluOpType.add)
                        lg = finp.tile([P, TB * H * KP], f32, tag="lg", bufs=1)   # [t, h, k]
                        nc.vector.tensor_tensor(
                            out=lg[:].rearrange("p (t h k) -> p t k h", t=TB, h=H)[:, :nt_],
                            in0=a_l[:].rearrange("p (t k h) -> p t k h", t=TB, k=KP)[:, :nt_],
                            in1=a_r[:].rearrange("p (t h) -> p t h", t=TB)[:, :nt_]
                                .unsqueeze(2).to_broadcast([P, nt_, KP, H]),
                            op=mybir.AluOpType.add)
                        lk2 = finp.tile([P, TB * H * KP], f32, tag="lk2", bufs=1)
                        nc.vector.tensor_scalar_mul(lk2[:], lg[:], NEG_SLOPE)
                        nc.vector.tensor_tensor(out=lk2[:], in0=lk2[:], in1=lg[:],
                                                op=mybir.AluOpType.max)
                        ex = finp.tile([P, TB * H * KP], f32, tag="ex", bufs=1)
                        nc.scalar.activation(out=ex[:], in_=lk2[:],
                                             func=mybir.ActivationFunctionType.Exp)
                        den = finp.tile([P, TB * H], f32, tag="den", bufs=1)
                        nc.vector.tensor_reduce(
                            out=den[:].rearrange("p (t h) -> p t h", t=TB),
                            in_=ex[:].rearrange("p (t h k) -> p t h k", t=TB, h=H),
                            axis=mybir.AxisListType.X, op=mybir.AluOpType.add)
                        rden = finp.tile([P, TB * H], f32, tag="rden", bufs=1)
                        nc.vector.reciprocal(out=rden[:], in_=den[:])
                        hopa = finp.tile([P, TB * H * KP], f32, tag="hopa", bufs=1)
                        nc.vector.tensor_tensor(
                            out=hopa[:].rearrange("p (t h k) -> p t h k", t=TB, h=H),
                            in0=ex[:].rearrange("p (t h k) -> p t h k", t=TB, h=H),
                            in1=rden[:].rearrange("p (t h) -> p t h", t=TB)
                                .unsqueeze(3).to_broadcast([P, TB, H, KP]),
                            op=mybir.AluOpType.mult)
                        pr2 = finp.tile([P, TB * HD * KP], f32, tag="pr2", bufs=1)
                        for tl in range(nt_):
                            nc.vector.tensor_tensor(
                                out=pr2[:].rearrange("p (t h d k) -> p t k h d",
                                                     t=TB, h=H, d=D)[:, tl],
                                in0=hst[:, tl].rearrange("p k (h d) -> p k h d", h=H),
                                in1=hopa[:].rearrange("p (t h k) -> p t k h",
                                                      t=TB, h=H)[:, tl]
                                    .unsqueeze(3).to_broadcast([P, KP, H, D]),
                                op=mybir.AluOpType.mult)
                        rst = finp.tile([P, TB * HD], f32, tag="rst", bufs=1)
                        nc.vector.tensor_reduce(
                            out=rst[:].rearrange("p (t e) -> p t e", t=TB)[:, :nt_],
                            in_=pr2[:].rearrange("p (t e k) -> p t e k", t=TB, k=KP)[:, :nt_],
                            axis=mybir.AxisListType.X, op=mybir.AluOpType.add)
                        nc.vector.tensor_tensor(
                            out=rst[:].rearrange("p (t e) -> p t e", t=TB)[:, :nt_],
                            in0=rst[:].rearrange("p (t e) -> p t e", t=TB)[:, :nt_],
                            in1=bias_s[:].unsqueeze(1).to_broadcast([P, nt_, HD]),
                            op=mybir.AluOpType.add)
                        for tl, t in enumerate(ts):
                            nc.sync.dma_start(
                                out=out_ext[t * P:(t + 1) * P, :],
                                in_=rst[:].rearrange("p (t e) -> p t e", t=TB)[:, tl, :])

            for k in range(1, K + 1):
                hop(k)

    nc.compile()
    return nc


# ---------------- entry point ----------------

def kernel(**inputs) -> np.ndarray:
    in_maps, tiles, batches, NSLOT, NCHUNK, NNMAX = _prep(**inputs)
    nc = _build(tiles, batches, NSLOT, NCHUNK, NNMAX)
    trace = bool(int(os.environ.get("AGDN_TRACE", "0")))
    res = run_bass_kernel_spmd(nc, in_maps, core_ids=list(range(NC)), trace=trace,
                               tmpdir=os.environ.get("AGDN_TMPDIR") or None)
    if trace:
        kernel.last_exec_time_ns = res.exec_time_ns
        kernel.last_res = res
    out = np.empty((N, H, D), np.float32)
    for c in range(NC):
        o = res.results[c]["out"]                    # [NLOCP, HD]
        out[c * NLOC:c * NLOC + HALF_DATA] = o[:HALF_DATA].reshape(-1, H, D)
        out[c * NLOC + HALF_DATA:(c + 1) * NLOC] = \
            o[HALF:HALF + HALF_DATA].reshape(-1, H, D)
    return out


# revision 18
# speedup vs baseline: 2.2413x; 1.4913x over previous
"""AGDNConv (GAT transition + K-hop diffusion + hop attention) on 8 TRN2 NeuronCores.

Strategy (dst-sharded graph parallel), v2 — gather-descriptor-bound redesign:
- Nodes sharded contiguously: core c owns nodes [c*6250, (c+1)*6250).
- Stage 1 (per core): fc matmul (H0^T = W @ feat^T), attention projections
  el/er; hop-1 table rows are [H0 | el | pad] bf16 (768B); er stays in SBUF.
- Per-hop node tables (bf16) are replicated via AllGather in two halves
  (A/B by local index) so hop k+1's half-A gathers overlap half-B's collective.
- Edge slots are grouped (half, dst-tile): per tile, chunks of 128 slots.
  The SWDGE dma_gather descriptor generation on GpSimd (~5.2ns/idx + ~2us/call)
  is the kernel bottleneck, so gathers are batched: one call per (half x
  3-tile batch), ~34 calls/hop, ~112k idx/hop.
- Aggregation: per 128-slot chunk, one PE matmul with a host-precomputed
  128-wide 0/1 selection matrix (fp8, DMA'd from DRAM) accumulating into the
  dst tile's PSUM. Pad slots have all-zero sel columns (no masking needed).
- Per-edge softmax weights w = exp(leaky(el[src]+er[dst])): el comes from the
  hop-1 gathered rows; er[dst] per slot is produced by a tiny PE matmul with
  the TRANSPOSED selection matrix (selT, also host-precomputed) against the
  tile's er values resident in SBUF -- this replaces the baseline's 1ms
  per-edge er gather. Softmax denominators ride as 4 extra w columns in the
  hop-1 aggregation matmul; every hop's PSUM is scaled by 1/s at drain.
- The hop-attention finale is fused into hop-3's tile loop (batched), so no
  serial tail.

All index manipulation (sharding, slot layout, sel matrices) happens on the
host; all floating-point math runs on device.
"""

import os
import numpy as np
import ml_dtypes

import concourse.bacc as bacc
import concourse.bass as bass
import concourse.mybir as mybir
import concourse.tile as tile
from concourse.bass_utils import run_bass_kernel_spmd
from concourse.masks import make_identity

# ---------------- problem geometry (hardcoded per spec) ----------------
N, E, IN, H, D, K = 50000, 800000, 256, 4, 64, 3
HD = H * D                     # 256
NC = 8
NLOC = N // NC                 # 6250
HALF_DATA = NLOC // 2          # 3125
HALF = 3200                    # padded rows per half (25 tiles of 128)
NLOCP = 2 * HALF               # 6400
NT = NLOCP // 128              # 50 dst tiles
NT_A = HALF // 128             # 25 (tiles 0..24 are half A)
TROWS = NC * HALF              # 25600 rows per half-table (int16-safe)
ELEM1 = 384                    # hop-1 table row elems (256 H0 + 4 el + pad)
ELEM = 256
NEG_SLOPE = 0.2
TB = 2                         # dst tiles per gather/compute batch

P = 128
f32 = mybir.dt.float32
bf16 = mybir.dt.bfloat16
fp8 = mybir.dt.float8e4
i16 = mybir.dt.int16


# ---------------- host-side preparation ----------------

def _prep(feat, src, dst, W_src, attn_l, attn_r, hop_attn_l, hop_attn_r,
          pos_emb, bias):
    src = np.asarray(src).astype(np.int64)
    dst = np.asarray(dst).astype(np.int64)
    feat = np.asarray(feat, dtype=np.float32)

    core = dst // NLOC
    j = dst % NLOC
    jp = j + (HALF - HALF_DATA) * (j >= HALF_DATA)  # padded local coord of dst
    t_of = jp >> 7
    dl = jp & 127

    sc = src // NLOC
    sj = src % NLOC
    shalf = (sj >= HALF_DATA).astype(np.int64)
    tblidx = sc * HALF + sj - HALF_DATA * shalf     # row within half-table

    # group = (src-half, dst-tile); slot space = [A:t0..t49 | B:t0..t49]
    NG = 2 * NT
    g_of = shalf * NT + t_of

    counts = np.zeros((NC, NG), np.int64)
    for c in range(NC):
        counts[c] = np.bincount(g_of[core == c], minlength=NG)
    nch_g = (np.max(counts, axis=0) + 127) // 128          # chunks per group
    cap_g = nch_g * 128
    gbase = np.zeros(NG + 1, np.int64)
    np.cumsum(cap_g, out=gbase[1:])
    NSLOT = int(gbase[-1])
    NCHUNK = NSLOT // 128

    # per-core slot arrays
    idx_main = np.zeros((NC, NSLOT), np.int16)   # pad slots read row 0 (killed by zero sel col)
    sel_in = np.zeros((NC, P, NCHUNK * 128), np.uint8)
    selT_in = np.zeros((NC, P, NCHUNK * 128), np.uint8)
    one_fp8 = np.array(1.0, dtype=ml_dtypes.float8_e4m3).view(np.uint8)

    for c in range(NC):
        m = np.nonzero(core == c)[0]
        o = m[np.argsort(g_of[m], kind="stable")]
        g_sorted = g_of[o]
        gstart = np.zeros(NG + 1, np.int64)
        np.cumsum(counts[c], out=gstart[1:])
        rank = np.arange(len(o)) - gstart[g_sorted]
        slot = gbase[g_sorted] + rank
        idx_main[c, slot] = tblidx[o]
        ch = slot >> 7
        sp = slot & 127
        d_ = dl[o]
        sel_in[c, sp, ch * 128 + d_] = one_fp8
        selT_in[c, d_, ch * 128 + sp] = one_fp8

    def wrap16(a):
        w = a.reshape(NSLOT // 16, 16).T            # [16, NSLOT//16]
        return np.tile(w, (8, 1)).copy()            # [128, NSLOT//16]

    # device-side schedule: per tile, chunk offsets; per batch, gather ranges
    tiles = []
    for t in range(NT):
        tiles.append({
            "cA0": int(gbase[t] // 128), "nA": int(nch_g[t]),
            "cB0": int(gbase[NT + t] // 128), "nB": int(nch_g[NT + t]),
        })
    batches = []
    for b0 in range(0, NT, TB):
        ts = list(range(b0, min(b0 + TB, NT)))
        cA0 = tiles[ts[0]]["cA0"]
        nnA = sum(tiles[t]["nA"] for t in ts)
        cB0 = tiles[ts[0]]["cB0"]
        nnB = sum(tiles[t]["nB"] for t in ts)
        batches.append({"tiles": ts, "cA0": cA0, "nnA": nnA,
                        "cB0": cB0, "nnB": nnB})
    NNMAX = max(b["nnA"] + b["nnB"] for b in batches)

    # feat^T padded per core
    featT = np.zeros((NC, IN, NLOCP), np.float32)
    for c in range(NC):
        fc = feat[c * NLOC:(c + 1) * NLOC]          # [6250, 256]
        featT[c, :, :HALF_DATA] = fc[:HALF_DATA].T
        featT[c, :, HALF:HALF + HALF_DATA] = fc[HALF_DATA:].T

    WT = np.ascontiguousarray(np.asarray(W_src, np.float32).T)    # [IN, HD]
    al = np.asarray(attn_l, np.float32).reshape(H, D)
    ar = np.asarray(attn_r, np.float32).reshape(H, D)
    AlAr = np.zeros((HD, 2 * H), np.float32)
    for h in range(H):
        AlAr[h * D:(h + 1) * D, h] = al[h]
        AlAr[h * D:(h + 1) * D, H + h] = ar[h]

    hopl = np.asarray(hop_attn_l, np.float32).reshape(H * D)
    hopr = np.asarray(hop_attn_r, np.float32).reshape(H * D)
    hopl_r = np.broadcast_to(hopl, (P, HD)).copy()
    hopr_r = np.broadcast_to(hopr, (P, HD)).copy()
    pos = np.asarray(pos_emb, np.float32).reshape(H, K + 1, D)
    pos_flat = np.transpose(pos, (1, 0, 2)).reshape(K + 1, HD)    # [k, h*64+d]
    pos_r = np.broadcast_to(pos_flat, (P, K + 1, HD)).copy()
    bias_r = np.broadcast_to(np.asarray(bias, np.float32).reshape(HD), (P, HD)).copy()

    in_maps = []
    for c in range(NC):
        in_maps.append({
            "featT": featT[c],
            "WT": WT,
            "AlAr": AlAr,
            "hopl": hopl_r,
            "hopr": hopr_r,
            "pos": pos_r,
            "bias": bias_r,
            "idxm": wrap16(idx_main[c]),
            "selin": sel_in[c].view(ml_dtypes.float8_e4m3),
            "selTin": selT_in[c].view(ml_dtypes.float8_e4m3),
        })
    return in_maps, tiles, batches, NSLOT, NCHUNK, NNMAX


# ---------------- device kernel ----------------

def _build(tiles, batches, NSLOT, NCHUNK, NNMAX):
    # 4 SWDGE queues: each dma_gather is processed by the Q7 core pair
    # (2q, 2q+1), so round-robining queue_num across calls runs descriptor
    # generation on all 8 DSPs in parallel.
    nqueues = int(os.environ.get("AGDN_NQ", "4"))
    nc = bacc.Bacc("TRN2", debug=False, num_swdge_queues=nqueues)

    featT_in = nc.dram_tensor("featT", [IN, NLOCP], f32, kind="ExternalInput")
    WT_in = nc.dram_tensor("WT", [IN, HD], f32, kind="ExternalInput")
    AlAr_in = nc.dram_tensor("AlAr", [HD, 2 * H], f32, kind="ExternalInput")
    hopl_in = nc.dram_tensor("hopl", [P, HD], f32, kind="ExternalInput")
    hopr_in = nc.dram_tensor("hopr", [P, HD], f32, kind="ExternalInput")
    pos_in = nc.dram_tensor("pos", [P, K + 1, HD], f32, kind="ExternalInput")
    bias_in = nc.dram_tensor("bias", [P, HD], f32, kind="ExternalInput")
    idxm_in = nc.dram_tensor("idxm", [P, NSLOT // 16], i16, kind="ExternalInput")
    selin_in = nc.dram_tensor("selin", [P, NCHUNK * 128], fp8, kind="ExternalInput")
    selTin_in = nc.dram_tensor("selTin", [P, NCHUNK * 128], fp8, kind="ExternalInput")
    out_ext = nc.dram_tensor("out", [NLOCP, HD], f32, kind="ExternalOutput")

    rg = [list(range(NC))]

    with tile.TileContext(nc) as tc:
        with (
            tc.tile_pool(name="dram", bufs=1, space="DRAM") as dram,
            tc.tile_pool(name="pers", bufs=1) as pers,
            tc.tile_pool(name="work", bufs=2) as work,
            tc.tile_pool(name="gat", bufs=2) as gat,
            tc.tile_pool(name="fin", bufs=2) as finp,
            tc.tile_pool(name="psum", bufs=2, space="PSUM") as psum,
            tc.tile_pool(name="apsum", bufs=2, space="PSUM") as apsum,
            tc.tile_pool(name="epsum", bufs=2, space="PSUM") as epsum,
        ):
            # ---- persistent DRAM ----
            shardA = [dram.tile([HALF, ELEM1 if k == 1 else ELEM], bf16,
                                tag=f"shA{k}", name=f"shA{k}") for k in (1, 2, 3)]
            shardB = [dram.tile([HALF, ELEM1 if k == 1 else ELEM], bf16,
                                tag=f"shB{k}", name=f"shB{k}") for k in (1, 2, 3)]
            tblA = [dram.tile([TROWS, ELEM1 if k == 1 else ELEM], bf16, addr_space="Shared",
                              tag=f"tbA{k}", name=f"tbA{k}") for k in (1, 2, 3)]
            tblB = [dram.tile([TROWS, ELEM1 if k == 1 else ELEM], bf16, addr_space="Shared",
                              tag=f"tbB{k}", name=f"tbB{k}") for k in (1, 2, 3)]
            hstack = dram.tile([NLOCP, K, HD], bf16, tag="hstack")

            # ---- persistent SBUF ----
            w_all = pers.tile([P, NCHUNK, H], bf16, tag="w")
            recip_all = pers.tile([P, NT, H], f32, tag="recip")
            er_all = pers.tile([P, NT, H], bf16, tag="erall")
            idxm = pers.tile([P, NSLOT // 16], i16, tag="idxm")
            hopl_s = pers.tile([P, HD], f32, tag="hopl")
            hopr_s = pers.tile([P, HD], f32, tag="hopr")
            pos_s = pers.tile([P, K + 1, HD], f32, tag="pos")
            bias_s = pers.tile([P, HD], f32, tag="bias")
            ident = pers.tile([P, P], bf16, tag="ident")

            nc.sync.dma_start(out=idxm[:], in_=idxm_in[:])
            nc.sync.dma_start(out=hopl_s[:], in_=hopl_in[:])
            nc.sync.dma_start(out=hopr_s[:], in_=hopr_in[:])
            nc.sync.dma_start(out=pos_s[:], in_=pos_in[:])
            nc.sync.dma_start(out=bias_s[:], in_=bias_in[:])
            make_identity(nc, ident[:])

            # ---- stage 1: fc + el/er + hop-1 table + hstack0 ----
            WT_s = pers.tile([P, 2, HD], f32, tag="wts")     # [k-chunk, o]
            AlAr_s = pers.tile([P, 2, 2 * H], f32, tag="alar")
            nc.sync.dma_start(out=WT_s[:], in_=WT_in[:].rearrange("(a p) o -> p a o", p=P))
            nc.sync.dma_start(out=AlAr_s[:], in_=AlAr_in[:].rearrange("(a p) o -> p a o", p=P))
            AlAr_bf = pers.tile([P, 2, 2 * H], bf16, tag="alarbf")
            nc.vector.tensor_copy(out=AlAr_bf[:], in_=AlAr_s[:])

            NB = 512

            def stage1_block(nb):
                n0 = nb * NB
                w_ = min(NB, NLOCP - n0)
                ft = [work.tile([P, NB], f32, tag=f"ft{i}", name=f"ft{i}") for i in range(2)]
                for kc in range(2):
                    nc.sync.dma_start(out=ft[kc][:, :w_],
                                      in_=featT_in[kc * P:(kc + 1) * P, n0:n0 + w_])
                h0t_sb = work.tile([P, 2, NB], bf16, tag="h0t")
                for oh in range(2):
                    h0t_ps = psum.tile([P, NB], f32, space="PSUM", tag="h0tp")
                    for kc in range(2):
                        nc.tensor.matmul(
                            h0t_ps[:, :w_],
                            lhsT=WT_s[:, kc, oh * P:(oh + 1) * P],
                            rhs=ft[kc][:, :w_],
                            start=(kc == 0), stop=(kc == 1),
                        )
                    nc.vector.tensor_copy(out=h0t_sb[:, oh, :w_], in_=h0t_ps[:, :w_])
                for sub in range(w_ // P):
                    t = (n0 + sub * P) // P
                    eler_ps = psum.tile([P, 2 * H], f32, space="PSUM", tag="elerp", bufs=1)
                    for oh in range(2):
                        nc.tensor.matmul(
                            eler_ps[:],
                            lhsT=h0t_sb[:, oh, sub * P:(sub + 1) * P],
                            rhs=AlAr_bf[:, oh, :],
                            start=(oh == 0), stop=(oh == 1),
                        )
                    h0row_ps = psum.tile([P, HD], bf16, space="PSUM", tag="h0rp", bufs=1)
                    for oh in range(2):
                        nc.tensor.transpose(
                            out=h0row_ps[:, oh * P:(oh + 1) * P],
                            in_=h0t_sb[:, oh, sub * P:(sub + 1) * P],
                            identity=ident[:],
                        )
                    row_sb = work.tile([P, ELEM1], bf16, tag="row1")
                    nc.vector.tensor_copy(out=row_sb[:, 0:HD], in_=h0row_ps[:])
                    nc.vector.tensor_copy(out=row_sb[:, HD:HD + H], in_=eler_ps[:, 0:H])
                    nc.vector.memset(row_sb[:, HD + H:], 0.0)
                    sh, r0 = (shardA[0], t * P) if t < NT_A else (shardB[0], (t - NT_A) * P)
                    nc.sync.dma_start(out=sh[r0:r0 + P, :], in_=row_sb[:])
                    # er for this tile stays resident in SBUF
                    nc.vector.tensor_copy(out=er_all[:, t, :], in_=eler_ps[:, H:2 * H])
                    # hstack k=0: H0 + pos[0]
                    hs0 = work.tile([P, HD], bf16, tag="hs")
                    nc.vector.tensor_tensor(out=hs0[:], in0=h0row_ps[:],
                                            in1=pos_s[:, 0, :], op=mybir.AluOpType.add)
                    nc.sync.dma_start(out=hstack[t * P:(t + 1) * P, 0, :], in_=hs0[:])

            for nb in range(NLOCP // NB + (1 if NLOCP % NB else 0)):
                stage1_block(nb)
                if nb * NB < HALF <= (nb + 1) * NB:
                    nc.gpsimd.collective_compute(
                        "AllGather", mybir.AluOpType.bypass, replica_groups=rg,
                        ins=[shardA[0].opt()], outs=[tblA[0].opt()])
            nc.gpsimd.collective_compute(
                "AllGather", mybir.AluOpType.bypass, replica_groups=rg,
                ins=[shardB[0].opt()], outs=[tblB[0].opt()])

            # ---- hops ----
            qrr = [0]   # round-robin SWDGE queue counter

            def hop(k):
                """k = 1,2,3: read tbl[k-1], write shard[k]/tbl[k] (k<3), hstack k.
                k==K folds the hop-attention finale into the tile loop."""
                el1 = ELEM1 if k == 1 else ELEM
                ncol = HD + H if k == 1 else HD
                tA, tB = tblA[k - 1], tblB[k - 1]
                for b in batches:
                    ts = b["tiles"]
                    cA0, nnA, cB0, nnB = b["cA0"], b["nnA"], b["cB0"], b["nnB"]
                    nn = nnA + nnB
                    g_flat = gat.tile([P, NNMAX * ELEM1], bf16, tag="g")
                    g = g_flat[:, 0:NNMAX * el1].rearrange("p (c s) -> p c s", s=el1)
                    gcap = int(os.environ.get("AGDN_GCAP", "8"))
                    for tbl_, c0_, o0_, nn_ in ((tA, cA0, 0, nnA),
                                                (tB, cB0, nnA, nnB)):
                        for b0_ in range(0, nn_, gcap):
                            n_ = min(gcap, nn_ - b0_)
                            nc.gpsimd.dma_gather(
                                g[:, o0_ + b0_:o0_ + b0_ + n_, :], tbl_[:],
                                idxm[:, (c0_ + b0_) * 8:(c0_ + b0_ + n_) * 8],
                                n_ * P, n_ * P, el1,
                                queue_num=qrr[0] % nqueues)
                            qrr[0] += 1
                    selb = gat.tile([P, NNMAX, 128], fp8, tag="selb")
                    nc.sync.dma_start(out=selb[:, 0:nnA, :],
                                      in_=selin_in[:, cA0 * 128:(cA0 + nnA) * 128]
                                      .rearrange("p (c s) -> p c s", s=128))
                    nc.sync.dma_start(out=selb[:, nnA:nn, :],
                                      in_=selin_in[:, cB0 * 128:(cB0 + nnB) * 128]
                                      .rearrange("p (c s) -> p c s", s=128))

                    # local chunk list per tile: (local idx in g/selb, tile)
                    loc_of_tile = {}
                    off = 0
                    for t in ts:
                        loc_of_tile[t] = {"A": (off, tiles[t]["nA"])}
                        off += tiles[t]["nA"]
                    for t in ts:
                        loc_of_tile[t]["B"] = (off, tiles[t]["nB"])
                        off += tiles[t]["nB"]

                    if k == 1:
                        selTb = gat.tile([P, NNMAX, 128], fp8, tag="selTb")
                        nc.sync.dma_start(out=selTb[:, 0:nnA, :],
                                          in_=selTin_in[:, cA0 * 128:(cA0 + nnA) * 128]
                                          .rearrange("p (c s) -> p c s", s=128))
                        nc.sync.dma_start(out=selTb[:, nnA:nn, :],
                                          in_=selTin_in[:, cB0 * 128:(cB0 + nnB) * 128]
                                          .rearrange("p (c s) -> p c s", s=128))
                        # er per slot via selT matmuls: er_ps[:, i*H:(i+1)*H]
                        er_ps = epsum.tile([P, NNMAX * H], f32, space="PSUM", tag="erps")
                        for t in ts:
                            for piece in ("A", "B"):
                                o0, cnt = loc_of_tile[t][piece]
                                for i in range(o0, o0 + cnt):
                                    nc.tensor.matmul(
                                        er_ps[:, i * H:(i + 1) * H],
                                        lhsT=selTb[:, i, :],
                                        rhs=er_all[:, t, :],
                                        start=True, stop=True)
                        # e = el + er ; w = exp(max(e, .2e))
                        e_f = work.tile([P, NNMAX, H], f32, tag="ef")
                        nc.vector.tensor_tensor(
                            out=e_f[:, :nn, :], in0=g[:, :nn, HD:HD + H],
                            in1=er_ps[:].rearrange("p (c h) -> p c h", h=H)[:, :nn, :],
                            op=mybir.AluOpType.add)
                        lk = work.tile([P, NNMAX, H], f32, tag="lk")
                        nc.vector.tensor_scalar_mul(lk[:, :nn, :], e_f[:, :nn, :], NEG_SLOPE)
                        nc.vector.tensor_tensor(
                            out=lk[:, :nn, :], in0=lk[:, :nn, :],
                            in1=e_f[:, :nn, :], op=mybir.AluOpType.max)
                        wb = work.tile([P, NNMAX, H], f32, tag="wb")
                        nc.scalar.activation(
                            out=wb[:, :nn, :], in_=lk[:, :nn, :],
                            func=mybir.ActivationFunctionType.Exp)
                        # persist w (bf16) for hops 2,3 at global chunk coords
                        nc.vector.tensor_copy(out=w_all[:, cA0:cA0 + nnA, :],
                                              in_=wb[:, 0:nnA, :])
                        nc.vector.tensor_copy(out=w_all[:, cB0:cB0 + nnB, :],
                                              in_=wb[:, nnA:nn, :])
                        # weight rows; write w into cols 256:260 for denominators
                        nc.vector.tensor_tensor(
                            out=g[:, :nn, 0:HD].rearrange("p c (h d) -> p c h d", h=H),
                            in0=g[:, :nn, 0:HD].rearrange("p c (h d) -> p c h d", h=H),
                            in1=wb[:, :nn, :].unsqueeze(3).to_broadcast([P, nn, H, D]),
                            op=mybir.AluOpType.mult)
                        nc.vector.tensor_copy(out=g[:, :nn, HD:HD + H], in_=wb[:, :nn, :])
                    else:
                        nc.vector.tensor_tensor(
                            out=g[:, 0:nnA, 0:HD].rearrange("p c (h d) -> p c h d", h=H),
                            in0=g[:, 0:nnA, 0:HD].rearrange("p c (h d) -> p c h d", h=H),
                            in1=w_all[:, cA0:cA0 + nnA, :].unsqueeze(3)
                                .to_broadcast([P, nnA, H, D]),
                            op=mybir.AluOpType.mult)
                        nc.vector.tensor_tensor(
                            out=g[:, nnA:nn, 0:HD].rearrange("p c (h d) -> p c h d", h=H),
                            in0=g[:, nnA:nn, 0:HD].rearrange("p c (h d) -> p c h d", h=H),
                            in1=w_all[:, cB0:cB0 + nnB, :].unsqueeze(3)
                                .to_broadcast([P, nnB, H, D]),
                            op=mybir.AluOpType.mult)

                    if k == K:
                        hst = finp.tile([P, TB, K + 1, HD], bf16, tag="hst")
                        for tl, t in enumerate(ts):
                            nc.sync.dma_start(
                                out=hst[:, tl, 0:K, :],
                                in_=hstack[t * P:(t + 1) * P, :, :])

                    # aggregation matmuls + drain per tile
                    for tl, t in enumerate(ts):
                        chunks = []
                        for piece in ("A", "B"):
                            o0, cnt = loc_of_tile[t][piece]
                            chunks.extend(range(o0, o0 + cnt))
                        ps = apsum.tile([P, 320], f32, space="PSUM", tag="agg")
                        for ci, i in enumerate(chunks):
                            nc.tensor.matmul(
                                ps[:, 0:ncol],
                                lhsT=selb[:, i, :],
                                rhs=g[:, i, 0:ncol],
                                start=(ci == 0), stop=(ci == len(chunks) - 1),
                            )
                        if k == 1:
                            s_eps = work.tile([P, H], f32, tag="seps")
                            nc.vector.tensor_scalar_add(s_eps[:], ps[:, HD:HD + H], 1e-30)
                            nc.vector.reciprocal(out=recip_all[:, t, :], in_=s_eps[:])
                        if k < K:
                            hk = work.tile([P, HD], bf16, tag="hk")
                            nc.vector.tensor_tensor(
                                out=hk[:].rearrange("p (h d) -> p h d", h=H),
                                in0=ps[:, 0:HD].rearrange("p (h d) -> p h d", h=H),
                                in1=recip_all[:, t, :].unsqueeze(2).to_broadcast([P, H, D]),
                                op=mybir.AluOpType.mult)
                            sh, r0 = (shardA[k], t * P) if t < NT_A else \
                                     (shardB[k], (t - NT_A) * P)
                            nc.sync.dma_start(out=sh[r0:r0 + P, :], in_=hk[:])
                            hs = work.tile([P, HD], bf16, tag="hs2")
                            nc.vector.tensor_tensor(out=hs[:], in0=hk[:],
                                                    in1=pos_s[:, k, :],
                                                    op=mybir.AluOpType.add)
                            nc.sync.dma_start(out=hstack[t * P:(t + 1) * P, k, :], in_=hs[:])
                        else:
                            # k==K: write h3+pos3 straight into the finale buffer
                            tmp = work.tile([P, HD], f32, tag="hk3f")
                            nc.vector.tensor_tensor(
                                out=tmp[:].rearrange("p (h d) -> p h d", h=H),
                                in0=ps[:, 0:HD].rearrange("p (h d) -> p h d", h=H),
                                in1=recip_all[:, t, :].unsqueeze(2).to_broadcast([P, H, D]),
                                op=mybir.AluOpType.mult)
                            nc.vector.tensor_tensor(out=hst[:, tl, K, :], in0=tmp[:],
                                                    in1=pos_s[:, K, :],
                                                    op=mybir.AluOpType.add)
                        if k < K:
                            if t == NT_A - 1:
                                nc.gpsimd.collective_compute(
                                    "AllGather", mybir.AluOpType.bypass, replica_groups=rg,
                                    ins=[shardA[k].opt()], outs=[tblA[k].opt()])
                            elif t == NT - 1:
                                nc.gpsimd.collective_compute(
                                    "AllGather", mybir.AluOpType.bypass, replica_groups=rg,
                                    ins=[shardB[k].opt()], outs=[tblB[k].opt()])

                    # ---- fused hop-attention finale for this batch ----
                    if k == K:
                        nt_ = len(ts)
                        KP = K + 1
                        prod = finp.tile([P, TB * KP * HD], f32, tag="prod", bufs=1)
                        nc.vector.tensor_tensor(
                            out=prod[:].rearrange("p (t k e) -> p t k e", t=TB, k=KP)[:, :nt_],
                            in0=hst[:, :nt_],
                            in1=hopl_s[:].unsqueeze(1).unsqueeze(1)
                                .to_broadcast([P, nt_, KP, HD]),
                            op=mybir.AluOpType.mult)
                        a_l = finp.tile([P, TB * KP * H], f32, tag="al", bufs=1)
                        for tl in range(nt_):
                            nc.vector.tensor_reduce(
                                out=a_l[:].rearrange("p (t k h) -> p t k h",
                                                     t=TB, k=KP)[:, tl],
                                in_=prod[:].rearrange("p (t k h d) -> p t k h d",
                                                      t=TB, k=KP, h=H)[:, tl],
                                axis=mybir.AxisListType.X, op=mybir.AluOpType.add)
                        prr = finp.tile([P, TB * HD], f32, tag="prr", bufs=1)
                        nc.vector.tensor_tensor(
                            out=prr[:].rearrange("p (t e) -> p t e", t=TB)[:, :nt_],
                            in0=hst[:, :nt_, 0, :],
                            in1=hopr_s[:].unsqueeze(1).to_broadcast([P, nt_, HD]),
                            op=mybir.AluOpType.mult)
                        a_r = finp.tile([P, TB * H], f32, tag="ar", bufs=1)
                        nc.vector.tensor_reduce(
                            out=a_r[:].rearrange("p (t h) -> p t h", t=TB)[:, :nt_],
                            in_=prr[:].rearrange("p (t h d) -> p t h d", t=TB, h=H)[:, :nt_],
                            axis=mybir.AxisListType.X, op=mybir.AluOpType.add)
                        lg = finp.tile([P, TB * H * KP], f32, tag="lg", bufs=1)   # [t, h, k]
                        nc.vector.tensor_tensor(
                            out=lg[:].rearrange("p (t h k) -> p t k h", t=TB, h=H)[:, :nt_],
                            in0=a_l[:].rearrange("p (t k h) -> p t k h", t=TB, k=KP)[:, :nt_],
                            in1=a_r[:].rearrange("p (t h) -> p t h", t=TB)[:, :nt_]
                                .unsqueeze(2).to_broadcast([P, nt_, KP, H]),
                            op=mybir.AluOpType.add)
                        lk2 = finp.tile([P, TB * H * KP], f32, tag="lk2", bufs=1)
                        nc.vector.tensor_scalar_mul(lk2[:], lg[:], NEG_SLOPE)
                        nc.vector.tensor_tensor(out=lk2[:], in0=lk2[:], in1=lg[:],
                                                op=mybir.AluOpType.max)
                        ex = finp.tile([P, TB * H * KP], f32, tag="ex", bufs=1)
                        nc.scalar.activation(out=ex[:], in_=lk2[:],
                                             func=mybir.ActivationFunctionType.Exp)
                        den = finp.tile([P, TB * H], f32, tag="den", bufs=1)
                        nc.vector.tensor_reduce(
                            out=den[:].rearrange("p (t h) -> p t h", t=TB),
                            in_=ex[:].rearrange("p (t h k) -> p t h k", t=TB, h=H),
                            axis=mybir.AxisListType.X, op=mybir.AluOpType.add)
                        rden = finp.tile([P, TB * H], f32, tag="rden", bufs=1)
                        nc.vector.reciprocal(out=rden[:], in_=den[:])
                        hopa = finp.tile([P, TB * H * KP], f32, tag="hopa", bufs=1)
                        nc.vector.tensor_tensor(
                            out=hopa[:].rearrange("p (t h k) -> p t h k", t=TB, h=H),
                            in0=ex[:].rearrange("p (t h k) -> p t h k", t=TB, h=H),
                            in1=rden[:].rearrange("p (t h) -> p t h", t=TB)
                                .unsqueeze(3).to_broadcast([P, TB, H, KP]),
                            op=mybir.AluOpType.mult)
                        pr2 = finp.tile([P, TB * HD * KP], f32, tag="pr2", bufs=1)
                        for tl in range(nt_):
                            nc.vector.tensor_tensor(
                                out=pr2[:].rearrange("p (t h d k) -> p t k h d",
                                                     t=TB, h=H, d=D)[:, tl],
                                in0=hst[:, tl].rearrange("p k (h d) -> p k h d", h=H),
                                in1=hopa[:].rearrange("p (t h k) -> p t k h",
                                                      t=TB, h=H)[:, tl]
                                    .unsqueeze(3).to_broadcast([P, KP, H, D]),
                                op=mybir.AluOpType.mult)
                        rst = finp.tile([P, TB * HD], f32, tag="rst", bufs=1)
                        nc.vector.tensor_reduce(
                            out=rst[:].rearrange("p (t e) -> p t e", t=TB)[:, :nt_],
                            in_=pr2[:].rearrange("p (t e k) -> p t e k", t=TB, k=KP)[:, :nt_],
                            axis=mybir.AxisListType.X, op=mybir.AluOpType.add)
                        nc.vector.tensor_tensor(
                            out=rst[:].rearrange("p (t e) -> p t e", t=TB)[:, :nt_],
                            in0=rst[:].rearrange("p (t e) -> p t e", t=TB)[:, :nt_],
                            in1=bias_s[:].unsqueeze(1).to_broadcast([P, nt_, HD]),
                            op=mybir.AluOpType.add)
                        for tl, t in enumerate(ts):
                            nc.sync.dma_start(
                                out=out_ext[t * P:(t + 1) * P, :],
                                in_=rst[:].rearrange("p (t e) -> p t e", t=TB)[:, tl, :])

            for k in range(1, K + 1):
                hop(k)

    nc.compile()
    return nc


# ---------------- entry point ----------------

def kernel(**inputs) -> np.ndarray:
    in_maps, tiles, batches, NSLOT, NCHUNK, NNMAX = _prep(**inputs)
    nc = _build(tiles, batches, NSLOT, NCHUNK, NNMAX)
    trace = bool(int(os.environ.get("AGDN_TRACE", "0")))
    res = run_bass_kernel_spmd(nc, in_maps, core_ids=list(range(NC)), trace=trace,
                               tmpdir=os.environ.get("AGDN_TMPDIR") or None)
    if trace:
        kernel.last_exec_time_ns = res.exec_time_ns
        kernel.last_res = res
    out = np.empty((N, H, D), np.float32)
    for c in range(NC):
        o = res.results[c]["out"]                    # [NLOCP, HD]
        out[c * NLOC:c * NLOC + HALF_DATA] = o[:HALF_DATA].reshape(-1, H, D)
        out[c * NLOC + HALF_DATA:(c + 1) * NLOC] = \
            o[HALF:HALF + HALF_DATA].reshape(-1, H, D)
    return out
